# revision 1
# baseline (speedup 1.0000x reference)
"""v4: split count pass (DVE fused + DVE compare->ACT accum), Newton-1
fused into the load, 3-step Newton, rank-based endgame length, fp16 masks.

Contract identical to kernel.py.  Per-mask threshold search:
  N1 count at tau0 piggybacks on the load (DVE idle there), N2/N3 Newton
  refine, two probe iterations establish a guaranteed bracket, then a
  short bisection endgame; masks written as fp16 {0,1} and converted to
  bool on host.  Counting pass: tiles 0..5 DVE is_ge -> fp16 ind -> ACT
  Identity accum; tiles 6..7 DVE fused is_ge+accum; per-row sum via one
  [128,128] PE matmul + tiny reduce.
"""

import sys
import functools
import numpy as np

sys.path.insert(0, "/opt/trn_rl_repo")

B, N, T = 128, 131072, 64
HW = N // T
N_CORES = 8
RPC = B // N_CORES
PPR = 128 // RPC
FD = N // PPR
NT = FD // HW
LO_INIT = -50.0
HI_INIT = -1e-5
LOG1E9 = float(np.log(np.float32(1e-9)))
# counting tiles: (offset, width, via_act) — ACT gets ~11k elems, DVE-fused
# the rest, so both engines finish together and the ACT tail stays short
CTILES = [(0, 4096, True), (4096, 4096, True), (8192, 4096, True),
          (12288, 4096, False)]
NC_T = len(CTILES)
RANK_TOL = 1.5


def _host_consts():
    lin = np.linspace(1.0, 0.001, T, dtype=np.float32)
    prefpow_t = np.power(lin, np.float32(1.0 / 3.0)).astype(np.float32)
    pre = np.zeros((128, NT), dtype=np.float32)
    for p in range(128):
        for j in range(NT):
            pre[p, j] = prefpow_t[(p % PPR) * NT + j]
    gm = np.zeros((128, 128), dtype=np.float32)
    for p in range(128):
        g = p // PPR
        gm[p, g * PPR:(g + 1) * PPR] = 1.0
    return pre, gm


@functools.lru_cache(maxsize=2)
def _newton_consts(k_src: int, k_tgt: int):
    """Distribution-derived (synthetic, fixed seed): tau0, inv-slope,
    bracket half-width delta after a 3-step Newton, endgame length."""
    rng = np.random.default_rng(987654321)
    R = 48
    clip = lambda u: np.clip(u, 1e-3, 1 - 1e-3).astype(np.float32)
    lin = np.linspace(1.0, 0.001, T, dtype=np.float32)
    L = np.power(lin, np.float32(1.0 / 3.0)).astype(np.float32)
    rep = lambda x: np.repeat(x, HW, axis=-1)

    U0s = clip(rng.random((R, N), dtype=np.float32))
    Uts = clip(rng.random((R, T), dtype=np.float32))
    U0t = clip(rng.random((R, N), dtype=np.float32))
    Utt = clip(rng.random((R, T), dtype=np.float32))
    P_src = (np.log(U0s) + np.log(rep(Uts)) / 2
             + np.log(rep(L[None, :].repeat(R, 0)))).astype(np.float32)
    src = P_src >= np.partition(P_src, N - k_src, axis=1)[:, N - k_src][:, None]
    P_tgt = (np.log(U0t) + np.log(rep(Utt)) / 2
             + np.where(src, np.float32(LOG1E9), np.float32(0.0))).astype(np.float32)

    out = []
    for P, k in ((P_src, k_src), (P_tgt, k_tgt)):
        tau_star = np.partition(P, N - k, axis=1)[:, N - k]
        tau0 = float(np.median(tau_star))
        h = 0.02
        cnt = lambda t: (P >= t).sum(axis=1).astype(np.float64)
        s = float((cnt(tau0 - h) - cnt(tau0 + h)).mean() / (2 * h))
        inv_s = 1.0 / s
        t_i = tau0 + (cnt(tau0) - k) * inv_s
        for _ in range(2):  # steps 2 and 3
            n_i = (P >= t_i[:, None]).sum(axis=1)
            t_i = t_i + (n_i - k) * inv_s
        resid = float(np.abs(t_i - tau_star).max())
        delta = max(3.0 * resid, 0.0015)
        n_end = int(np.ceil(np.log2(max(2 * delta * s / RANK_TOL, 4.0))))
        out.append((tau0, inv_s, delta, n_end))
    return out


@functools.lru_cache(maxsize=4)
def _build(k_src: int, k_tgt: int):
    import concourse.bass as bass
    import concourse.tile as tile
    from concourse import bacc, mybir
    from concourse.alu_op_type import AluOpType as op
    from contextlib import ExitStack

    f32 = mybir.dt.float32
    f16 = mybir.dt.float16
    u8 = mybir.dt.uint8
    AF = mybir.ActivationFunctionType
    (nw_s, nw_t) = _newton_consts(k_src, k_tgt)

    nc = bacc.Bacc("TRN2", target_bir_lowering=False, debug=False,
                   num_devices=N_CORES)

    u0s = nc.dram_tensor("u0s", [RPC, N], f32, kind="ExternalInput")
    uts = nc.dram_tensor("uts", [RPC, T], f32, kind="ExternalInput")
    u0t = nc.dram_tensor("u0t", [RPC, N], f32, kind="ExternalInput")
    utt = nc.dram_tensor("utt", [RPC, T], f32, kind="ExternalInput")
    pre_d = nc.dram_tensor("prefpow", [128, NT], f32, kind="ExternalInput")
    gm_d = nc.dram_tensor("groupm", [128, 128], f32, kind="ExternalInput")
    ms_d = nc.dram_tensor("ms", [RPC, N], f16, kind="ExternalOutput")
    mt_d = nc.dram_tensor("mt", [RPC, N], f16, kind="ExternalOutput")

    with tile.TileContext(nc) as tc, ExitStack() as ctx:
        pool = ctx.enter_context(tc.tile_pool(name="big", bufs=1))
        stage = ctx.enter_context(tc.tile_pool(name="stage", bufs=3))
        indp = ctx.enter_context(tc.tile_pool(name="indp", bufs=3))
        psum = ctx.enter_context(tc.tile_pool(name="ps", bufs=1, space="PSUM"))

        P = pool.tile([128, FD], f32, tag="P")
        SCR = pool.tile([128, FD], f32, tag="SCR")
        TRA = pool.tile([128, 4096], f16, tag="TRA")  # ACT accum main-out
        TR = pool.tile([128, 4096], f16, tag="TR")    # fused-count main-out
        GM = pool.tile([128, 128], f32, tag="GM")
        PRE = pool.tile([128, NT], f32, tag="PRE")
        SC8s = pool.tile([128, NT], f32, tag="SC8s")
        SC8t = pool.tile([128, NT], f32, tag="SC8t")
        UT8s = pool.tile([128, NT], f32, tag="UT8s")
        UT8t = pool.tile([128, NT], f32, tag="UT8t")
        CNT8 = pool.tile([128, NT], f32, tag="CNT8")
        xlos = pool.tile([128, 1], f32, tag="xlos")
        xlo = pool.tile([128, 1], f32, tag="xlo")
        xhi = pool.tile([128, 1], f32, tag="xhi")
        mid = pool.tile([128, 1], f32, tag="mid")
        tau = pool.tile([128, 1], f32, tag="tau")
        d1 = pool.tile([128, 1], f32, tag="d1")
        ssum = pool.tile([128, 1], f32, tag="ssum")
        cnts = pool.tile([128, 1], f32, tag="cnts")
        t0 = pool.tile([128, 1], u8, tag="t0")
        t1 = pool.tile([128, 1], u8, tag="t1")

        nc.sync.dma_start(GM[:], gm_d.ap())
        nc.sync.dma_start(PRE[:], pre_d.ap())

        def count_tile(SRC, j, thr_ap):
            """Emit count of SRC counting-tile j vs thr into CNT8[:, j]."""
            off, w, via_act = CTILES[j]
            sl = slice(off, off + w)
            if via_act:
                it = indp.tile([128, 4096], f16, tag="it")
                nc.vector.tensor_scalar(it[:, :w], SRC[:, sl], thr_ap, None,
                                        op0=op.is_ge)
                nc.scalar.activation(TRA[:, :w], it[:, :w], AF.Identity,
                                     accum_out=CNT8[:, j:j + 1])
            else:
                nc.vector.tensor_scalar(TR[:, :w], SRC[:, sl], thr_ap, None,
                                        op0=op.is_ge, op1=op.add,
                                        accum_out=CNT8[:, j:j + 1])

        def count_finish():
            ps8 = psum.tile([128, NC_T], f32, tag="ps8")
            nc.tensor.matmul(ps8[:], GM[:], CNT8[:, 0:NC_T], start=True,
                             stop=True)
            nc.vector.tensor_reduce(cnts[:], ps8[:], axis=mybir.AxisListType.X,
                                    op=op.add)

        def count(SRC, thr_ap):
            for j in range(NC_T):
                count_tile(SRC, j, thr_ap)
            count_finish()

        def newton_step(K, inv_s):
            nc.vector.tensor_scalar(d1[:], cnts[:], float(K), None,
                                    op0=op.subtract)
            nc.vector.tensor_scalar(d1[:], d1[:], float(inv_s), None,
                                    op0=op.mult)
            nc.vector.tensor_add(tau[:], tau[:], d1[:])

        def update(K):
            nc.vector.tensor_scalar(t0[:], cnts[:], float(K), None, op0=op.is_ge)
            nc.vector.copy_predicated(xlo[:], t0[:], mid[:])
            nc.vector.tensor_scalar(t1[:], cnts[:], float(K), None, op0=op.is_lt)
            nc.vector.copy_predicated(xhi[:], t1[:], mid[:])

        def load(u0_dram, ut_dram, SC8, UT8, DST, with_prefix, tau0):
            """Load + Ln; also piggyback the Newton-1 count at tau0."""
            ut_r = ut_dram.ap().rearrange("r (jp t) -> (r jp) t", jp=PPR)
            nc.sync.dma_start(UT8[:], ut_r)
            nc.scalar.activation(SC8[:], UT8[:], AF.Sqrt)
            if with_prefix:
                nc.vector.tensor_mul(SC8[:], SC8[:], PRE[:])
            nc.vector.memset(tau[:], tau0)
            u0_r = u0_dram.ap().rearrange("r (jp f) -> (r jp) f", jp=PPR)
            for j in range(NT):
                sl = slice(j * HW, (j + 1) * HW)
                st = stage.tile([128, HW], f32, tag="stg")
                nc.sync.dma_start(st[:], u0_r[:, sl])
                nc.scalar.activation(DST[:, sl], st[:], AF.Ln,
                                     scale=SC8[:, j:j + 1])

        def topk(SRC, K, consts, name, n1_done):
            tau0, inv_s, delta, n_end = consts
            with nc.named_scope(f"topk_{name}"):
                if not n1_done:
                    nc.vector.memset(tau[:], tau0)
                    count(SRC, tau[:])
                else:
                    # N1 counts were emitted tile-by-tile after the load
                    count_finish()
                newton_step(K, inv_s)
                for _ in range(2):          # Newton 2, 3
                    count(SRC, tau[:])
                    newton_step(K, inv_s)
                nc.vector.memset(xlo[:], LO_INIT)
                nc.vector.memset(xhi[:], HI_INIT)
                for sgn in (-1.0, 1.0):     # probes: always-valid bracket
                    nc.vector.tensor_scalar(mid[:], tau[:], sgn * delta, None,
                                            op0=op.add)
                    count(SRC, mid[:])
                    update(K)
                for _ in range(n_end):
                    nc.vector.tensor_add(ssum[:], xlo[:], xhi[:])
                    nc.vector.tensor_scalar_mul(mid[:], ssum[:], 0.5)
                    count(SRC, mid[:])
                    update(K)

        def maskout(SRC, out_dram, name):
            o_r = out_dram.ap().rearrange("r (jp f) -> (r jp) f", jp=PPR)
            with nc.named_scope(f"mask_{name}"):
                for j in range(4):
                    sl = slice(j * 4096, (j + 1) * 4096)
                    it = indp.tile([128, 4096], f16, tag="it")
                    nc.vector.tensor_scalar(it[:], SRC[:, sl], xlo[:], None,
                                            op0=op.is_ge)
                    nc.sync.dma_start(o_r[:, sl], it[:])

        tau0_s = nw_s[0]
        with nc.named_scope("load_src"):
            load(u0s, uts, SC8s, UT8s, P, True, tau0_s)
            for j in range(NC_T):          # free Newton-1 counts on DVE
                count_tile(P, j, tau[:])
        with nc.named_scope("load_tgt"):
            load(u0t, utt, SC8t, UT8t, SCR, False, nw_t[0])
            # load() resets tau; src N1 counts in CNT8 were already taken.

        nc.vector.memset(tau[:], tau0_s)
        topk(P, k_src, nw_s, "src", n1_done=True)
        nc.vector.tensor_copy(xlos[:], xlo[:])   # snapshot src threshold
        maskout(P, ms_d, "src")
        with nc.named_scope("penalty"):
            # SCR += log(1e-9) * (P >= xlo_src), via a two-valued fp16
            # product tile (exact: the product is a shared constant)
            for j in range(4):
                sl = slice(j * 4096, (j + 1) * 4096)
                nc.vector.tensor_scalar(TR[:, :4096], P[:, sl], xlos[:], LOG1E9,
                                        op0=op.is_ge, op1=op.mult)
                nc.vector.tensor_add(SCR[:, sl], SCR[:, sl], TR[:, :4096])
        topk(SCR, k_tgt, nw_t, "tgt", n1_done=False)
        maskout(SCR, mt_d, "tgt")

    nc.compile()
    return nc


def _in_maps(U0_src, Ut_src, U0_tgt, Ut_tgt):
    pre, gm = _host_consts()
    maps = []
    for c in range(N_CORES):
        rs = slice(c * RPC, (c + 1) * RPC)
        maps.append({
            "u0s": np.ascontiguousarray(U0_src[rs]),
            "uts": np.ascontiguousarray(Ut_src[rs]),
            "u0t": np.ascontiguousarray(U0_tgt[rs]),
            "utt": np.ascontiguousarray(Ut_tgt[rs]),
            "prefpow": pre,
            "groupm": gm,
        })
    return maps


def run(U0_src, Ut_src, U0_tgt, Ut_tgt, K_src, K_tgt, trace=False,
        trace_kwargs=None):
    import time
    from concourse.bass_utils import run_bass_kernel_spmd
    nc = _build(int(K_src), int(K_tgt))
    maps = _in_maps(np.asarray(U0_src, np.float32), np.asarray(Ut_src, np.float32),
                    np.asarray(U0_tgt, np.float32), np.asarray(Ut_tgt, np.float32))
    try:
        res = run_bass_kernel_spmd(nc, maps, list(range(N_CORES)), trace=trace,
                                   **(trace_kwargs or {}))
    except Exception:
        # transient NRT exec-unit failures have been observed; retry once
        time.sleep(15)
        res = run_bass_kernel_spmd(nc, maps, list(range(N_CORES)), trace=trace,
                                   **(trace_kwargs or {}))
    src = np.concatenate([res.results[c]["ms"] for c in range(N_CORES)], axis=0)
    tgt = np.concatenate([res.results[c]["mt"] for c in range(N_CORES)], axis=0)
    return (src != 0, tgt != 0), res


def kernel(U0_src, Ut_src, U0_tgt, Ut_tgt, K_src, K_tgt):
    (src, tgt), _ = run(U0_src, Ut_src, U0_tgt, Ut_tgt, K_src, K_tgt)
    return (src, tgt)



# revision 2
# speedup vs baseline: 3.2916x; 3.2916x over previous
"""v5: per-row analytic tau0 (host, from Ut only) + 3-count Newton on
fp16 threshold-centered P; tgt via masked counts (no penalty pass).

Pipeline per core (16 rows as [128 part x 16384 free], slot-contiguous):
  load src tiles -> ACT Ln(U0*SC') -> fp16 P16s (SC'=sqrt(Ut)*L*e^-tau0
  folds the slot constant AND the analytic threshold into the Ln scale,
  so the working value is P - tau0, threshold starts at 0); DVE counts
  N1 per tile during the load.  Newton x2 -> src mask m16 (its accum is
  the exact src count).  tgt identically Ln-centered at q0 = tau0_tgt -
  log(1e-9); because log(1e-9) separates the penalized groups strictly,
  tgt = ~src OR (src AND base>=th), and counts over src tokens only are
  scalar_tensor_tensor (is_ge, mult m16) accums.  Final tgt mask is one
  STT (is_ge, max 1-m).  Masks out as fp16 {0,1}; host compares != 0.
"""

import sys
import functools
import numpy as np

sys.path.insert(0, "/opt/trn_rl_repo")

B, N, T = 128, 131072, 64
HW = N // T
N_CORES = 8
RPC = B // N_CORES          # rows per core
PPR = 128 // RPC            # partitions per row
FD = N // PPR               # free dim per partition
NT = FD // HW               # slots per partition
EPS = 1e-3
LOG1E9 = float(np.log(np.float32(1e-9)))
TW = 2048                   # load/count tile width
NTILES = FD // TW


# ---------------- host analytics (Ut + K only) ----------------

def _surv(x):
    return np.where(x <= EPS, 1.0, np.where(x > 1 - EPS, 0.0, 1.0 - x))


def _solve_tau(c, K, lo, hi, iters=70):
    lo = np.full(c.shape[0], lo)
    hi = np.full(c.shape[0], hi)
    for _ in range(iters):
        mid = 0.5 * (lo + hi)
        cnt = (HW * _surv(np.exp(mid[:, None] - c))).sum(1)
        hi = np.where(cnt > K, hi, mid)
        lo = np.where(cnt > K, mid, lo)
    return 0.5 * (lo + hi)


def _host_analytics(Ut_src, Ut_tgt, K_src, K_tgt):
    """Per-row tau0/inverse-slope + per-slot Ln scales, from Ut only."""
    L = np.linspace(1.0, 0.001, T, dtype=np.float32) ** np.float32(1.0 / 3.0)
    cs = np.log(Ut_src.astype(np.float64)) / 2 + np.log(L.astype(np.float64))[None]
    ct = np.log(Ut_tgt.astype(np.float64)) / 2
    tau0_s = _solve_tau(cs, K_src, -15.0, 0.0)
    x = np.exp(tau0_s[:, None] - cs)
    act = (x > EPS) & (x <= 1 - EPS)
    inv_s = 1.0 / (HW * x * act).sum(1)
    ms = HW * _surv(x)                       # expected src tokens per slot
    # tgt threshold sits inside the penalized (src) group: solve mixture
    assert K_tgt > N - K_src + 4000, "masked-count scheme needs K_tgt > N-K_src"
    lo = np.full(B, -35.0)
    hi = np.full(B, 0.0)
    for _ in range(70):
        mid = 0.5 * (lo + hi)
        cnt = ((HW - ms) * _surv(np.exp(mid[:, None] - ct))
               + ms * _surv(np.exp(mid[:, None] - LOG1E9 - ct))).sum(1)
        hi = np.where(cnt > K_tgt, hi, mid)
        lo = np.where(cnt > K_tgt, mid, lo)
    tau0_t = 0.5 * (lo + hi)
    q0 = tau0_t - LOG1E9                      # base-space center
    xt = np.exp(q0[:, None] - ct)
    actt = (xt > EPS) & (xt <= 1 - EPS)
    inv_t = 1.0 / (ms * xt * actt).sum(1)
    SCs = np.exp(cs - tau0_s[:, None]).astype(np.float32)   # [B,T]
    SCt = np.exp(ct - q0[:, None]).astype(np.float32)       # [B,T]
    return SCs, SCt, inv_s.astype(np.float32), inv_t.astype(np.float32)


def _per_core_consts(SCs, SCt, inv_s, inv_t, core):
    """Rearrange [B,*] host constants into this core's [128,*] layout:
    partition p <- (row p//PPR, sub jp=p%PPR); slot s = jp*NT + j."""
    rs = slice(core * RPC, (core + 1) * RPC)
    scs_c, sct_c = SCs[rs], SCt[rs]           # [RPC, T]
    invs_c, invt_c = inv_s[rs], inv_t[rs]     # [RPC]
    scp_s = np.zeros((128, NT), dtype=np.float32)
    scp_t = np.zeros((128, NT), dtype=np.float32)
    ivs = np.zeros((128, 1), dtype=np.float32)
    ivt = np.zeros((128, 1), dtype=np.float32)
    for p in range(128):
        r, jp = p // PPR, p % PPR
        scp_s[p] = scs_c[r, jp * NT:(jp + 1) * NT]
        scp_t[p] = sct_c[r, jp * NT:(jp + 1) * NT]
        ivs[p, 0] = invs_c[r]
        ivt[p, 0] = invt_c[r]
    gm = np.zeros((128, 128), dtype=np.float32)
    for p in range(128):
        g = p // PPR
        gm[p, g * PPR:(g + 1) * PPR] = 1.0
    return scp_s, scp_t, ivs, ivt, gm


# ---------------- device kernel ----------------

@functools.lru_cache(maxsize=4)
def _build(k_src: int, k_tgt: int):
    import concourse.bass as bass
    import concourse.tile as tile
    from concourse import bacc, mybir
    from concourse.alu_op_type import AluOpType as op
    from contextlib import ExitStack

    f32 = mybir.dt.float32
    f16 = mybir.dt.float16
    AF = mybir.ActivationFunctionType

    nc = bacc.Bacc("TRN2", target_bir_lowering=False, debug=False,
                   num_devices=N_CORES)

    u0s = nc.dram_tensor("u0s", [RPC, N], f32, kind="ExternalInput")
    u0t = nc.dram_tensor("u0t", [RPC, N], f32, kind="ExternalInput")
    scps_d = nc.dram_tensor("scps", [128, NT], f32, kind="ExternalInput")
    scpt_d = nc.dram_tensor("scpt", [128, NT], f32, kind="ExternalInput")
    ivs_d = nc.dram_tensor("ivs", [128, 1], f32, kind="ExternalInput")
    ivt_d = nc.dram_tensor("ivt", [128, 1], f32, kind="ExternalInput")
    gm_d = nc.dram_tensor("groupm", [128, 128], f32, kind="ExternalInput")
    ms_d = nc.dram_tensor("ms", [RPC, N], f16, kind="ExternalOutput")
    mt_d = nc.dram_tensor("mt", [RPC, N], f16, kind="ExternalOutput")

    with tile.TileContext(nc) as tc, ExitStack() as ctx:
        pool = ctx.enter_context(tc.tile_pool(name="big", bufs=1))
        stage = ctx.enter_context(tc.tile_pool(name="stage", bufs=3))
        outp = ctx.enter_context(tc.tile_pool(name="outp", bufs=3))
        psum = ctx.enter_context(tc.tile_pool(name="ps", bufs=1, space="PSUM"))

        P16s = pool.tile([128, FD], f16, tag="P16s")
        P16t = pool.tile([128, FD], f16, tag="P16t")
        M16 = pool.tile([128, FD], f16, tag="M16")    # src mask (f16 0/1)
        INV16 = pool.tile([128, FD], f16, tag="INV16")  # 1 - src mask
        JNK = pool.tile([128, FD], f16, tag="JNK")    # count-pass outputs
        GM = pool.tile([128, 128], f32, tag="GM")
        SCPS = pool.tile([128, NT], f32, tag="SCPS")
        SCPT = pool.tile([128, NT], f32, tag="SCPT")
        IVS = pool.tile([128, 1], f32, tag="IVS")
        IVT = pool.tile([128, 1], f32, tag="IVT")
        CNTS = pool.tile([128, NTILES], f32, tag="CNTS")
        CNTT = pool.tile([128, NTILES], f32, tag="CNTT")
        C1 = pool.tile([128, 1], f32, tag="C1")
        D1 = pool.tile([128, 1], f32, tag="D1")
        TAU = pool.tile([128, 1], f32, tag="TAU")
        TH = pool.tile([128, 1], f32, tag="TH")
        CNT = pool.tile([128, 1], f32, tag="CNT")
        NSRC = pool.tile([128, 1], f32, tag="NSRC")
        KBASE = pool.tile([128, 1], f32, tag="KBASE")

        nc.sync.dma_start(GM[:], gm_d.ap())
        nc.sync.dma_start(SCPS[:], scps_d.ap())
        nc.sync.dma_start(SCPT[:], scpt_d.ap())
        nc.sync.dma_start(IVS[:], ivs_d.ap())
        nc.sync.dma_start(IVT[:], ivt_d.ap())
        nc.vector.memset(TAU[:], 0.0)
        nc.vector.memset(TH[:], 0.0)

        u0s_r = u0s.ap().rearrange("r (jp f) -> (r jp) f", jp=PPR)
        u0t_r = u0t.ap().rearrange("r (jp f) -> (r jp) f", jp=PPR)
        ms_r = ms_d.ap().rearrange("r (jp f) -> (r jp) f", jp=PPR)
        mt_r = mt_d.ap().rearrange("r (jp f) -> (r jp) f", jp=PPR)

        # ---- src load: DMA -> Ln (threshold-centered) -> fp16; N1 count per tile
        with nc.named_scope("load_src"):
            for j in range(NTILES):
                sl = slice(j * TW, (j + 1) * TW)
                st = stage.tile([128, TW], f32, tag="stg")
                nc.sync.dma_start(st[:], u0s_r[:, sl])
                nc.scalar.activation(P16s[:, sl], st[:], AF.Ln,
                                     scale=SCPS[:, j:j + 1])
                nc.vector.tensor_scalar(JNK[:, sl], P16s[:, sl], 0.0, None,
                                        op0=op.is_ge, op1=op.add,
                                        accum_out=CNTS[:, j:j + 1])

        # ---- tgt load: DMA -> Ln centered at q0 -> fp16
        with nc.named_scope("load_tgt"):
            for j in range(NTILES):
                sl = slice(j * TW, (j + 1) * TW)
                st = stage.tile([128, TW], f32, tag="stg")
                nc.sync.dma_start(st[:], u0t_r[:, sl])
                nc.scalar.activation(P16t[:, sl], st[:], AF.Ln,
                                     scale=SCPT[:, j:j + 1])

        def rowsum(cols_ap, ncols, out_ap):
            """per-row (groups of PPR partitions) sum, broadcast to [128,1]."""
            ps = psum.tile([128, ncols], f32, tag="ps")
            nc.tensor.matmul(ps[:], GM[:], cols_ap, start=True, stop=True)
            if ncols > 1:
                nc.vector.tensor_reduce(out_ap, ps[:], axis=mybir.AxisListType.X,
                                        op=op.add)
            else:
                nc.vector.tensor_copy(out_ap, ps[:])

        def newton(tau_ap, cnt_ap, k_ap_or_imm, inv_ap):
            # tau += (cnt - K) * inv
            if isinstance(k_ap_or_imm, float):
                nc.vector.tensor_scalar(D1[:], cnt_ap, k_ap_or_imm, None,
                                        op0=op.subtract)
            else:
                nc.vector.tensor_sub(D1[:], cnt_ap, k_ap_or_imm)
            nc.vector.tensor_mul(D1[:], D1[:], inv_ap)
            nc.vector.tensor_add(tau_ap, tau_ap, D1[:])

        # ---- src Newton chain
        with nc.named_scope("topk_src"):
            rowsum(CNTS[:, 0:NTILES], NTILES, CNT[:])
            newton(TAU[:], CNT[:], float(k_src), IVS[:])
            nc.vector.tensor_scalar(JNK[:], P16s[:], TAU[:], None,
                                    op0=op.is_ge, op1=op.add,
                                    accum_out=C1[:])
            rowsum(C1[:], 1, CNT[:])
            newton(TAU[:], CNT[:], float(k_src), IVS[:])
            # final src mask (+ exact count via accum)
            nc.vector.tensor_scalar(M16[:], P16s[:], TAU[:], None,
                                    op0=op.is_ge, op1=op.add,
                                    accum_out=C1[:])
            nc.sync.dma_start(ms_r[:, :], M16[:])
            rowsum(C1[:], 1, NSRC[:])
            # masked-count target: KBASE = n_src - (N - K_tgt)
            nc.vector.tensor_scalar(KBASE[:], NSRC[:], float(N - k_tgt), None,
                                    op0=op.subtract)
            nc.vector.tensor_scalar(INV16[:], P16s[:], TAU[:], None,
                                    op0=op.is_lt)

        # ---- tgt: masked counts over src tokens
        with nc.named_scope("topk_tgt"):
            for j in range(NTILES):
                sl = slice(j * TW, (j + 1) * TW)
                nc.vector.scalar_tensor_tensor(
                    JNK[:, sl], P16t[:, sl], 0.0, M16[:, sl],
                    op0=op.is_ge, op1=op.mult,
                    accum_out=CNTT[:, j:j + 1])
            rowsum(CNTT[:, 0:NTILES], NTILES, CNT[:])
            newton(TH[:], CNT[:], KBASE[:], IVT[:])
            nc.vector.scalar_tensor_tensor(JNK[:], P16t[:], TH[:], M16[:],
                                           op0=op.is_ge, op1=op.mult,
                                           accum_out=C1[:])
            rowsum(C1[:], 1, CNT[:])
            newton(TH[:], CNT[:], KBASE[:], IVT[:])
            # final: tgt = max(1 - m, base >= th), tiled with DMA chasing
            for j in range(NTILES):
                sl = slice(j * TW, (j + 1) * TW)
                ot = outp.tile([128, TW], f16, tag="ot")
                nc.vector.scalar_tensor_tensor(ot[:], P16t[:, sl], TH[:],
                                               INV16[:, sl],
                                               op0=op.is_ge, op1=op.max)
                nc.sync.dma_start(mt_r[:, sl], ot[:])

    nc.compile()
    return nc


def _in_maps(U0_src, Ut_src, U0_tgt, Ut_tgt, K_src, K_tgt):
    SCs, SCt, inv_s, inv_t = _host_analytics(Ut_src, Ut_tgt, K_src, K_tgt)
    maps = []
    for c in range(N_CORES):
        scp_s, scp_t, ivs, ivt, gm = _per_core_consts(SCs, SCt, inv_s, inv_t, c)
        rs = slice(c * RPC, (c + 1) * RPC)
        maps.append({
            "u0s": np.ascontiguousarray(U0_src[rs]),
            "u0t": np.ascontiguousarray(U0_tgt[rs]),
            "scps": scp_s,
            "scpt": scp_t,
            "ivs": ivs,
            "ivt": ivt,
            "groupm": gm,
        })
    return maps


def run(U0_src, Ut_src, U0_tgt, Ut_tgt, K_src, K_tgt, trace=False,
        trace_kwargs=None):
    import time
    from concourse.bass_utils import run_bass_kernel_spmd
    nc = _build(int(K_src), int(K_tgt))
    maps = _in_maps(np.asarray(U0_src, np.float32), np.asarray(Ut_src, np.float32),
                    np.asarray(U0_tgt, np.float32), np.asarray(Ut_tgt, np.float32),
                    int(K_src), int(K_tgt))
    try:
        res = run_bass_kernel_spmd(nc, maps, list(range(N_CORES)), trace=trace,
                                   **(trace_kwargs or {}))
    except Exception:
        # transient NRT exec-unit failures have been observed; retry once
        time.sleep(15)
        res = run_bass_kernel_spmd(nc, maps, list(range(N_CORES)), trace=trace,
                                   **(trace_kwargs or {}))
    src = np.concatenate([res.results[c]["ms"] for c in range(N_CORES)], axis=0)
    tgt = np.concatenate([res.results[c]["mt"] for c in range(N_CORES)], axis=0)
    return (src != 0, tgt != 0), res


def kernel(U0_src, Ut_src, U0_tgt, Ut_tgt, K_src, K_tgt):
    (src, tgt), _ = run(U0_src, Ut_src, U0_tgt, Ut_tgt, K_src, K_tgt)
    return (src, tgt)
